# revision 8
# baseline (speedup 1.0000x reference)
"""MoE4Embedder Trainium2 kernel.

Full-input contract: kernel(**inputs) takes the unsharded numpy inputs and
returns the full [32, 500, 512] f32 output. Internally shards tokens
(B*T = 16000) across 8 NeuronCores (2000 tokens each, padded to 2048);
expert/router weights are replicated.

Math (per token t with value v, x = gene_embedded[t]):
  h      = relu(x @ W1.T)              # [512]
  logits = h @ W2.T                    # [10]
  w      = softmax(logits)             # [10]
  sparse = w * (w >= fifth_largest(w)) # top-5 kept, rest zeroed
  out    = v * (shared_w.sum(0) + sparse @ routing_w)

Implementation notes:
- x is transposed host-side so the kernel streams xT tiles [d_chunk, tok]
  straight from DRAM (no on-chip input transposes).
- Matmuls run in float32r (fp32 with 11-bit mantissa, full PE rate at
  N>=512; plain fp32 runs at 1/4 rate). Inputs are pre-rounded on host.
- f32r logit error (~3e-4) can flip the top-5 selection for tokens whose
  5th/6th softmax weights are nearly tied. The kernel outputs each
  token's (m5, m6) = 5th/6th largest exp(logit); the host recomputes the
  few at-risk tokens (relative gap < RISK_THRESH, ~1%) in exact fp32.
- `value` multiplies the output via a per-partition scalar at the
  PSUM->SBUF copy, so it stays exact f32. The shared-expert row rides in
  the weighted-sum matmul with coefficient 1.0.
"""

import sys

sys.path.insert(0, "/opt/trn_rl_repo")

import numpy as np

B, T, D = 32, 500, 512
E = 10  # routing experts
EA = 11  # + shared-sum row
TOPK = 5
NCORE = 8
TPC = (B * T) // NCORE  # tokens per core = 2000
TPAD = 2048  # padded tokens per core
NG = 4  # groups of 512 tokens
GS = 512
P = 128

RISK_THRESH = 5e-3  # relative (m5-m6)/m5 gap below which host recomputes

_cache = {}


def _round_f32r(a):
    """Round-to-nearest f32 -> f32r (11-bit mantissa, low 12 bits zero)."""
    u = np.ascontiguousarray(a, np.float32).view(np.uint32)
    u = ((u + 0x800) & np.uint32(0xFFFFF000)).astype(np.uint32)
    return u.view(np.float32)


def _build_nc(mm_dt="float32r"):
    from concourse import bacc, mybir, tile, masks

    f32 = mybir.dt.float32
    mdt = getattr(mybir.dt, mm_dt)
    AF = mybir.ActivationFunctionType
    ALU = mybir.AluOpType
    AX = mybir.AxisListType

    nc = bacc.Bacc("TRN2", target_bir_lowering=False, debug=False)

    xtg_d = nc.dram_tensor("xtg", [NG, P, 4, GS], mdt, kind="ExternalInput")
    w1t_d = nc.dram_tensor("w1t", [P, 4, D], mdt, kind="ExternalInput")
    w2t_d = nc.dram_tensor("w2t", [P, 4, E], mdt, kind="ExternalInput")
    waug_d = nc.dram_tensor("waug", [EA, D], mdt, kind="ExternalInput")
    val_d = nc.dram_tensor("val", [P, TPAD // P], f32, kind="ExternalInput")
    out_d = nc.dram_tensor("out", [TPAD, D], f32, kind="ExternalOutput")
    gap_d = nc.dram_tensor("gap", [NG, P, 4, 2], f32, kind="ExternalOutput")

    with tile.TileContext(nc) as tc:
        with (
            tc.tile_pool(name="const", bufs=1) as cpool,
            tc.tile_pool(name="work", bufs=2) as wpool,
            tc.tile_pool(name="small", bufs=2) as spool,
            tc.tile_pool(name="outp", bufs=3) as opool,
            tc.tile_pool(name="ps_ht", bufs=1, space="PSUM") as ps_ht,
            tc.tile_pool(name="ps_lg", bufs=1, space="PSUM") as ps_lg,
            tc.tile_pool(name="ps_sw", bufs=1, space="PSUM") as ps_sw,
            tc.tile_pool(name="ps_out", bufs=2, space="PSUM") as ps_out,
        ):
            w1t = cpool.tile([P, 4, D], mdt)
            nc.sync.dma_start(out=w1t, in_=w1t_d[:])
            w2t = cpool.tile([P, 4, E], mdt)
            nc.sync.dma_start(out=w2t, in_=w2t_d[:])
            waug = cpool.tile([EA, D], mdt)
            nc.sync.dma_start(out=waug, in_=waug_d[:])
            val = cpool.tile([P, TPAD // P], f32)
            nc.sync.dma_start(out=val, in_=val_d[:])

            ident_f = cpool.tile([P, P], f32)
            masks.make_identity(nc, ident_f)
            ident = cpool.tile([P, P], mdt)
            nc.vector.tensor_copy(ident, ident_f)
            negbig = cpool.tile([P, 4, E], f32)
            nc.gpsimd.memset(negbig, -1e30)
            ones = cpool.tile([P, 4, 1], f32)
            nc.gpsimd.memset(ones, 1.0)

            for g in range(NG):
                # ---- load xT for this 512-token group ----
                xt = wpool.tile([P, 4, GS], mdt, tag="xt")
                nc.sync.dma_start(out=xt, in_=xtg_d[g])

                # ---- mm1: hT[e, tok] = relu(W1T.T @ xT), accumulate over d ----
                ht_ps = ps_ht.tile([P, 4, GS], f32)
                for e in range(4):
                    for k in range(4):
                        nc.tensor.matmul(
                            ht_ps[:, e, :],
                            w1t[:, k, P * e : P * (e + 1)],
                            xt[:, k, :],
                            start=(k == 0),
                            stop=(k == 3),
                        )
                ht = wpool.tile([P, 4, GS], mdt, tag="ht")
                for e in range(4):
                    nc.scalar.activation(ht[:, e, :], ht_ps[:, e, :], AF.Relu)

                # ---- mm2: logits[tok, e10] per 128-token subtile ----
                lg_ps = ps_lg.tile([P, 4, E], f32)
                for t4 in range(4):
                    for k in range(4):
                        nc.tensor.matmul(
                            lg_ps[:, t4, :],
                            ht[:, k, P * t4 : P * (t4 + 1)],
                            w2t[:, k, :],
                            start=(k == 0),
                            stop=(k == 3),
                        )

                # ---- softmax numerator + denominator (logits are O(1), no
                # max-subtraction needed for stability) ----
                exps = spool.tile([P, 4, E], f32, tag="exps")
                sums = spool.tile([P, 4], f32, tag="sums")
                for t4 in range(4):
                    nc.scalar.activation(
                        exps[:, t4, :],
                        lg_ps[:, t4, :],
                        AF.Exp,
                        accum_out=sums[:, t4 : t4 + 1],
                    )

                # ---- top-5 threshold: iteratively find max and knock it out.
                # Reduce #5 = m5 (keep threshold), reduce #6 = m6; (m5, m6)
                # are exported so the host can re-check near-ties in fp32. ----
                s = spool.tile([P, 4, E], f32, tag="s")
                nc.vector.tensor_copy(s, exps)
                m = spool.tile([P, 4, 1], f32, tag="m")
                gap = spool.tile([P, 4, 2], f32, tag="gap")
                mask = spool.tile([P, 4, E], f32, tag="mask")
                mask_i = spool.tile([P, 4, E], mybir.dt.int8, tag="mask_i")
                for it in range(6):
                    if it < 4:
                        red_out = m[:, :, 0]
                    else:
                        red_out = gap[:, :, it - 4]
                    nc.vector.tensor_reduce(red_out, s, axis=AX.X, op=ALU.max)
                    if it < 5:
                        if it == 4:
                            bc = gap[:, :, 0:1].broadcast_to([P, 4, E])
                        else:
                            bc = m.broadcast_to([P, 4, E])
                        nc.vector.tensor_tensor(mask_i, s, bc, op=ALU.is_ge)
                        nc.vector.copy_predicated(s, mask_i, negbig)
                nc.sync.dma_start(out=gap_d[g], in_=gap)

                # ---- sparse weights / sum, shared row coeff 1.0 ----
                nc.vector.tensor_tensor(
                    mask,
                    exps,
                    gap[:, :, 0:1].broadcast_to([P, 4, E]),
                    op=ALU.is_ge,
                )
                nc.vector.tensor_mul(exps, exps, mask)
                rs = spool.tile([P, 4, 1], f32, tag="rs")
                nc.vector.reciprocal(rs[:, :, 0], sums)
                swaug = spool.tile([P, 4, EA], mdt, tag="swaug")
                nc.vector.tensor_tensor(
                    swaug[:, :, 0:E], exps, rs.broadcast_to([P, 4, E]), op=ALU.mult
                )
                nc.vector.tensor_copy(swaug[:, :, E : E + 1], ones)

                # ---- mm3: out[tok, d] = swaugT.T @ Waug, then scale by value
                # (per-partition scalar) on the PSUM->SBUF copy ----
                for t4 in range(4):
                    swt_ps = ps_sw.tile([EA, P], mdt, tag="swt_ps")
                    nc.tensor.transpose(swt_ps, swaug[:, t4, :], ident)
                    swt = spool.tile([EA, P], mdt, tag="swt")
                    nc.vector.tensor_copy(swt, swt_ps)
                    o_ps = ps_out.tile([P, D], f32, tag="o_ps")
                    nc.tensor.matmul(o_ps, swt, waug, start=True, stop=True)
                    o_sb = opool.tile([P, D], f32, tag="o")
                    vcol = val[:, 4 * g + t4 : 4 * g + t4 + 1]
                    if t4 % 2 == 0:
                        nc.vector.tensor_scalar_mul(o_sb, o_ps, vcol)
                    else:
                        nc.scalar.activation(o_sb, o_ps, AF.Copy, scale=vcol)
                    row = GS * g + P * t4
                    nc.sync.dma_start(out=out_d[row : row + P, :], in_=o_sb)

    nc.compile()
    return nc


def _prep_inputs(gene_embedded, value, shared_w, routing_w, router_w1, router_w2):
    """Host-side shard + layout prep. Returns one in_map per core."""
    x = np.asarray(gene_embedded, np.float32).reshape(B * T, D)
    v = np.asarray(value, np.float32).reshape(B * T)

    w1t = _round_f32r(
        np.ascontiguousarray(
            np.asarray(router_w1, np.float32).T.reshape(4, P, D).transpose(1, 0, 2)
        )
    )  # [128, 4(dk), 512(e)]
    w2t = _round_f32r(
        np.ascontiguousarray(
            np.asarray(router_w2, np.float32).T.reshape(4, P, E).transpose(1, 0, 2)
        )
    )  # [128, 4(dk), 10]
    waug = np.zeros((EA, D), np.float32)
    waug[:E] = np.asarray(routing_w, np.float32)
    waug[E] = np.asarray(shared_w, np.float32).sum(axis=0)
    waug = _round_f32r(waug)

    in_maps = []
    for i in range(NCORE):
        xs = x[i * TPC : (i + 1) * TPC]
        xpad = np.zeros((TPAD, D), np.float32)
        xpad[:TPC] = xs
        # xtg[g, p, k, t] = xpad[512g + t, 128k + p]
        xtg = _round_f32r(
            np.ascontiguousarray(xpad.T.reshape(4, P, NG, GS).transpose(2, 1, 0, 3))
        )
        vpad = np.zeros(TPAD, np.float32)
        vpad[:TPC] = v[i * TPC : (i + 1) * TPC]
        v2d = np.ascontiguousarray(vpad.reshape(TPAD // P, P).T)
        in_maps.append(
            {"xtg": xtg, "w1t": w1t, "w2t": w2t, "waug": waug, "val": v2d}
        )
    return in_maps


def _host_patch(out, gaps, x, v, shared_w, routing_w, router_w1, router_w2):
    """Recompute tokens whose 5th/6th softmax weights are nearly tied.

    gaps: [B*T, 2] = (m5, m6) per token, core-concatenated.
    out:  [B*T, D] kernel output (modified in place).
    """
    m5, m6 = gaps[:, 0], gaps[:, 1]
    risk = (m5 - m6) <= RISK_THRESH * m5
    idx = np.nonzero(risk)[0]
    if idx.size == 0:
        return 0
    xs = x[idx]
    h = np.maximum(xs @ router_w1.T, 0.0)
    logits = h @ router_w2.T
    ex = np.exp(logits - logits.max(-1, keepdims=True))
    w = ex / ex.sum(-1, keepdims=True)
    thresh = np.sort(w, axis=-1)[:, E - TOPK][:, None]
    sparse = np.where(w >= thresh, w, 0.0)
    out[idx] = v[idx, None] * (shared_w.sum(0)[None, :] + sparse @ routing_w)
    return idx.size


def kernel(gene_embedded, value, shared_w, routing_w, router_w1, router_w2):
    from concourse.bass_utils import run_bass_kernel_spmd

    if "nc" not in _cache:
        _cache["nc"] = _build_nc()
    nc = _cache["nc"]

    in_maps = _prep_inputs(
        gene_embedded, value, shared_w, routing_w, router_w1, router_w2
    )
    res = run_bass_kernel_spmd(nc, in_maps, core_ids=list(range(NCORE)))
    _cache["last_result"] = res

    out = np.concatenate([r["out"][:TPC] for r in res.results], axis=0)
    # gap[g, p, t4, c] -> token-ordered [TPAD, 2]: token = 512g + 128 t4 + p
    gaps = np.concatenate(
        [
            np.asarray(r["gap"]).transpose(0, 2, 1, 3).reshape(TPAD, 2)[:TPC]
            for r in res.results
        ],
        axis=0,
    )
    x = np.asarray(gene_embedded, np.float32).reshape(B * T, D)
    v = np.asarray(value, np.float32).reshape(B * T)
    npatch = _host_patch(
        out, gaps, x, v,
        np.asarray(shared_w, np.float32),
        np.asarray(routing_w, np.float32),
        np.asarray(router_w1, np.float32),
        np.asarray(router_w2, np.float32),
    )
    _cache["npatch"] = npatch
    return np.ascontiguousarray(out.reshape(B, T, D))
